# revision 33
# baseline (speedup 1.0000x reference)
"""Trainium2 Bass kernel: dense transformer attention layer, TP over heads on 8 cores.

Strategy:
  - Shard the 32 heads across 8 cores (4 heads / core). wq/wk/wv column-sharded,
    wo row-sharded; x replicated (transposed + bf16-cast on host).
  - RoPE handled by permuting wq/wk rows on the host into a half-split layout so
    the on-device rotation touches contiguous partition blocks.
  - Fine-grained pipeline: each 512-token projection block is immediately
    followed by its attention chunk (all 4 heads). K is written by RoPE straight
    into resident SBUF tiles, V is copied from PSUM into a resident tile, and Q
    stays in SBUF too (q4 staging tiles double-buffered) - no DRAM round-trips.
  - Attention in transposed layout ST = K^T-major. Score tiles are computed in
    PAIRS into a 2-bank PSUM tile, so one ScalarE exp instruction covers 1024
    columns (halves Act instruction overhead - Act is the attention-phase
    bottleneck). The causal mask is a 0/1 DVE multiply on the exp'd pair
    (USE_BIAS_MM=True instead folds it pre-exp via an I.T @ bias matmul).
    Exp pairs accumulate on the DVE into S; a fold + ones-vector matmul forms
    row sums; the normalizer broadcast is a PE matmul (ones_row x rinv) whose
    PSUM bank is shared with the row-sum (WAR-ordered), freeing a bank for a
    third AV-accumulator buffer.
  - y (bf16) is exchanged in half-batch AllGather chunks, except the final
    chunk which is split per 512-token q block so the last collective is
    0.5 MB/rank and the post-collective tail is one 27us projection slice.
  - Output projection consumes gathered chunks in arrival order, emitted as
    small units; the last attention chunk pulls one unit per head so its
    Act-bound stretches are filled with independent PE work. Output is bf16,
    cast to fp32 on the host.
Timing: `_run_timed` pipelines k executions asynchronously (per-core NEFF
executions serialize on-device) and reports the marginal per-execution time
(T_k2 - T_k1)/(k2 - k1), which cancels the ~70-100 ms axon RPC round-trip.
"""

import sys
import math
import numpy as np

for _p in ("/opt/trn_rl_repo",):
    if _p not in sys.path:
        sys.path.insert(0, _p)

import ml_dtypes  # noqa: E402

import concourse.bass as bass  # noqa: E402
import concourse.mybir as mybir  # noqa: E402
import concourse.tile as tile  # noqa: E402
from concourse import bacc  # noqa: E402
from concourse.bass_utils import run_bass_kernel_spmd  # noqa: E402

BF16 = mybir.dt.bfloat16
F32 = mybir.dt.float32
BF16NP = ml_dtypes.bfloat16

B, L, NH, HD = 2, 2048, 32, 128
C = NH * HD              # 4096
T = B * L                # 4096 tokens total
NCORES = 8
DPC = C // NCORES        # 512 dims per core
HPC = DPC // HD          # 4 heads per core
FO = C // 128            # 32 feature blocks (contraction)
TN1 = 512                # token block for projection phases
NB1 = T // TN1           # 8
QBS = 512                # q block for attention
QB = L // QBS            # 4 per batch
KTILES = L // 128        # 16 k tiles per batch
SCALE = 1.0 / math.sqrt(HD)
MASK_BIAS = -1024.0      # pre-scale additive bias; exp(SCALE*(s-1024)) == 0
# True: apply causal mask as a PE bias-accumulate matmul (I.T @ bias) before
# exp. False: multiply the exp'd pair by a 0/1 mask tile on the DVE.
USE_BIAS_MM = False

_CACHED = {}


def _classify(maskT_bool):
    """cls[kt, qb]: 0 skip, 1 mixed (needs bias), 2 full."""
    cls = np.zeros((KTILES, QB), np.int8)
    pats = {}          # pattern bytes -> index
    pat_of = {}        # (kt, qb) -> pattern index
    for kt in range(KTILES):
        for qb in range(QB):
            m = maskT_bool[kt * 128:(kt + 1) * 128, qb * QBS:(qb + 1) * QBS]
            cls[kt, qb] = 0 if not m.any() else (2 if m.all() else 1)
            if cls[kt, qb] == 1:
                key = m.tobytes()
                if key not in pats:
                    pats[key] = len(pats)
                pat_of[(kt, qb)] = pats[key]
    npat = max(1, len(pats))
    bias = np.zeros((128, npat, QBS), np.float32)
    for key, idx in pats.items():
        m = np.frombuffer(key, bool).reshape(128, QBS)
        bias[:, idx, :] = (np.where(m, 0.0, MASK_BIAS) if USE_BIAS_MM
                           else m.astype(np.float32))
    return cls, pat_of, bias.astype(BF16NP)


def _build(maskT_bool, dist=True):
    """maskT_bool: [L, L] bool, maskT[k, q] = attend(q -> k)."""
    cls, pat_of, bias_np = _classify(maskT_bool)
    npat = bias_np.shape[1]

    nc = bacc.Bacc("TRN2", target_bir_lowering=False, debug=False,
                   num_devices=NCORES)

    xt = nc.dram_tensor("xt", [C, T], BF16, kind="ExternalInput")
    wqk_d = nc.dram_tensor("wqk", [2 * HPC, 128, FO * 128], BF16,
                           kind="ExternalInput")
    wv_d = nc.dram_tensor("wv", [128, FO * DPC], BF16, kind="ExternalInput")
    wo4_d = nc.dram_tensor("wo4", [HPC, 128, FO * 128], BF16,
                           kind="ExternalInput")
    cs_d = nc.dram_tensor("cs2", [128, 2, T], BF16, kind="ExternalInput")
    bias_d = nc.dram_tensor("biasm", [128, npat * QBS], BF16,
                            kind="ExternalInput")
    ident_d = nc.dram_tensor("ident", [128, 128], BF16, kind="ExternalInput")
    ones_d = nc.dram_tensor("ones", [128, 1], BF16, kind="ExternalInput")
    out_d = nc.dram_tensor("out", [DPC, T], BF16, kind="ExternalOutput")

    Exp = mybir.ActivationFunctionType.Exp

    with tile.TileContext(nc) as tc, nc.allow_low_precision(
            reason="bf16 rope temps / softmax-normalizer broadcast / bf16 "
                   "output; rel-err budget is 2e-2 and matmul accumulation "
                   "stays fp32"):
        with (
            tc.tile_pool(name="stage", bufs=2) as stage,
            tc.tile_pool(name="psum", bufs=1, space="PSUM") as psp,
            tc.tile_pool(name="dram", bufs=1, space="DRAM") as dram,
        ):
            # y is exchanged in half-batch chunks (2 q blocks = 1024 tokens),
            # except the LAST chunk which is split per-q-block so the final
            # collective is small and the post-collective tail is a single
            # 27us projection slice
            groups = []           # (b, [qb...])
            for b in range(B):
                for h in range(QB // 2):
                    qbs = [2 * h, 2 * h + 1]
                    if b == B - 1 and h == QB // 2 - 1:
                        groups.append((b, [qbs[0]]))
                        groups.append((b, [qbs[1]]))
                    else:
                        groups.append((b, qbs))
            grp_of = {(b, qb): gi for gi, (b, qbs) in enumerate(groups)
                      for qb in qbs}
            y_loc = [dram.tile([DPC, len(qbs) * QBS], BF16,
                               name=f"y_loc{gi}")
                     for gi, (b, qbs) in enumerate(groups)]
            y_full = [dram.tile([C, len(qbs) * QBS], BF16,
                                addr_space="Shared", name=f"y_full{gi}")
                      for gi, (b, qbs) in enumerate(groups)]
            xt_r = xt.rearrange("(fo p) t -> p fo t", p=128)

            with (
                tc.tile_pool(name="wres", bufs=1) as wres,
                tc.tile_pool(name="xs", bufs=5) as xsp,
                tc.tile_pool(name="qy", bufs=2) as qyp,
                tc.tile_pool(name="ptp", bufs=3) as ptp,
            ):
                # ---- phase 1: QKV projection + RoPE
                # emission order = DMA FIFO order: interleave the first token
                # block's x chunks with the weight tiles so the first matmul
                # group starts after ~2MB of DMA, not 16MB
                GF = 8            # fo per x chunk
                NG = FO // GF     # 4 chunks per token block

                def load_x(n):
                    tsl = slice(n * TN1, (n + 1) * TN1)
                    xc = []
                    for g in range(NG):
                        xg = xsp.tile([128, GF, TN1], BF16, tag="xchunk",
                                      name=f"xg{n}_{g}")
                        nc.sync.dma_start(
                            xg[:], xt_r[:, g * GF:(g + 1) * GF, tsl])
                        xc.append(xg)
                    cs_sb = stage.tile([128, 2, TN1], BF16, tag="csl", bufs=2,
                                       name=f"cs{n}")
                    nc.sync.dma_start(cs_sb[:], cs_d[:, :, tsl])
                    return xc, cs_sb

                # prologue DMA order: the first pair-group's matmul chain
                # needs x chunks 0..3 + wmb0 (+ cs0 for rope, wmb1 for the
                # second half) - load exactly that first, everything else after
                w_mb = []
                x0c = []
                x0g0 = xsp.tile([128, GF, TN1], BF16, tag="xchunk", name="xg0_0")
                nc.sync.dma_start(x0g0[:], xt_r[:, 0:GF, 0:TN1])
                x0c.append(x0g0)
                t0 = wres.tile([128, FO, 128], BF16, name="wmb0")
                nc.sync.dma_start(t0[:], wqk_d[0].rearrange(
                    "p (fo j) -> p fo j", j=128))
                w_mb.append(t0)
                for g in range(1, NG):
                    xg = xsp.tile([128, GF, TN1], BF16, tag="xchunk",
                                  name=f"xg0_{g}")
                    nc.sync.dma_start(
                        xg[:], xt_r[:, g * GF:(g + 1) * GF, 0:TN1])
                    x0c.append(xg)
                cs0 = stage.tile([128, 2, TN1], BF16, tag="csl",
                                 bufs=2, name="cs0")
                nc.sync.dma_start(cs0[:], cs_d[:, :, 0:TN1])
                for mb in range(1, 2 * HPC):
                    t = wres.tile([128, FO, 128], BF16, name=f"wmb{mb}")
                    nc.sync.dma_start(t[:], wqk_d[mb].rearrange(
                        "p (fo j) -> p fo j", j=128))
                    w_mb.append(t)
                w_v = wres.tile([128, FO, DPC], BF16)
                nc.sync.dma_start(w_v[:], wv_d.rearrange("p (fo j) -> p fo j", j=DPC))
                ones_sb = wres.tile([128, 1], BF16)
                nc.sync.dma_start(ones_sb[:], ones_d[:, :])
                bias_sb = wres.tile([128, npat, QBS], BF16)
                nc.sync.dma_start(bias_sb[:], bias_d.rearrange(
                    "p (np q) -> p np q", q=QBS))
                ident_sb = wres.tile([128, 128], BF16)
                nc.sync.dma_start(ident_sb[:], ident_d[:, :])
                # [1,128] all-ones lhsT used to broadcast the softmax
                # normalizer across partitions on the PE
                ones_row = wres.tile([1, 128], BF16)
                nc.vector.memset(ones_row[:], 1.0)
                # K and V stay SBUF-resident for the current batch
                k_res = [wres.tile([128, L], BF16, name=f"kres{hb}")
                         for hb in range(HPC)]
                v_res = wres.tile([128, KTILES, DPC], BF16)

                q4 = {}    # block n -> q staging tile [128, HPC, TN1]

                def phase1_block(n):
                    b, j = divmod(n, NB1 // B)
                    wtsl = slice(j * TN1, (j + 1) * TN1)
                    if n == 0:
                        xc, cs_sb = x0c, cs0
                    else:
                        xc, cs_sb = load_x(n)
                    cos_sb = cs_sb[:, 0]
                    sin_sb = cs_sb[:, 1]
                    q4[n] = qyp.tile([128, HPC, TN1], BF16, tag="q4",
                                     name=f"q4_{n}")
                    # q/k head-blocks in PSUM pairs (2 banks per tile)
                    for mbp in range(HPC):
                        ps = psp.tile([128, 2, TN1], F32, tag="mmp", bufs=2)
                        for half in range(2):
                            mb = 2 * mbp + half
                            for fo in range(FO):
                                nc.tensor.matmul(ps[:, half], w_mb[mb][:, fo],
                                                 xc[fo // GF][:, fo % GF],
                                                 start=(fo == 0),
                                                 stop=(fo == FO - 1))
                        for half in range(2):
                            mb = 2 * mbp + half
                            # rope: out = p*cos2 + rot(p)*sin2 (top half of
                            # sin2 negated on host)
                            tmp = stage.tile([128, TN1], BF16, tag="ropetmp", bufs=1)
                            rot = stage.tile([128, TN1], BF16, tag="roperot", bufs=1)
                            nc.vector.tensor_mul(tmp[:], ps[:, half], cos_sb)
                            nc.vector.tensor_mul(rot[0:64], ps[64:128, half],
                                                 sin_sb[0:64])
                            nc.vector.tensor_mul(rot[64:128], ps[0:64, half],
                                                 sin_sb[64:128])
                            if mb < HPC:   # Q head-block: into SBUF staging
                                nc.vector.tensor_add(q4[n][:, mb], tmp[:],
                                                     rot[:])
                            else:          # K head-block: into k_res
                                nc.vector.tensor_add(
                                    k_res[mb - HPC][:, wtsl], tmp[:], rot[:])
                    for tb in range(TN1 // 128):
                        psv = psp.tile([128, DPC], F32, tag="acc", bufs=3)
                        for fo in range(FO):
                            nc.tensor.matmul(
                                psv[:],
                                xc[fo // GF][:, fo % GF, tb * 128:(tb + 1) * 128],
                                w_v[:, fo], start=(fo == 0), stop=(fo == FO - 1))
                        nc.any.tensor_copy(v_res[:, j * 4 + tb, :], psv[:])

                # ---- attention chunk (b, qb): all 4 heads for one 512-token
                # q block; score tiles in pairs, one exp per pair, causal mask
                # as a 0/1 DVE multiply (or bias-accumulate matmul). `filler`
                # is an optional generator pulled once per head to interleave
                # independent PE work (phase-3 units) into the emission
                def attn_chunk(b, qb, filler=None):
                    n = b * (NB1 // B) + qb
                    acts = [kt for kt in range(KTILES) if cls[kt, qb] > 0]
                    npair_ = len(acts) // 2
                    rem = len(acts) - 2 * npair_
                    y4 = qyp.tile([128, HPC, QBS], BF16, tag="y4", bufs=1,
                                  name=f"y4_{b}_{qb}")
                    for hb in range(HPC):
                        if filler is not None:
                            next(filler, None)
                        hsl = slice(hb * 128, (hb + 1) * 128)
                        y_ps = psp.tile([128, QBS], F32, tag="acc", bufs=3)
                        S = ptp.tile([128, 2, QBS], BF16, tag="ssum", bufs=2)
                        nmm = npair_ + rem
                        if rem and nmm == 1:
                            nc.vector.memset(S[:, 1], 0.0)
                        for ip in range(nmm):
                            pair = [acts[2 * ip]] if (rem and ip == nmm - 1) \
                                else acts[2 * ip:2 * ip + 2]
                            st = psp.tile([128, 2, QBS], F32, tag="mmp",
                                          bufs=2)
                            for half, kt in enumerate(pair):
                                mixed = USE_BIAS_MM and cls[kt, qb] == 1
                                nc.tensor.matmul(
                                    st[:, half],
                                    k_res[hb][:, kt * 128:(kt + 1) * 128],
                                    q4[n][:, hb], start=True, stop=not mixed)
                                if mixed:
                                    nc.tensor.matmul(
                                        st[:, half], ident_sb[:],
                                        bias_sb[:, pat_of[(kt, qb)]],
                                        start=False, stop=True)
                            nhalf = len(pair)
                            dst = S if ip == 0 else ptp.tile(
                                [128, 2, QBS], BF16, tag="pt", bufs=2)
                            nc.scalar.activation(dst[:, 0:nhalf],
                                                 st[:, 0:nhalf], Exp,
                                                 scale=SCALE)
                            if not USE_BIAS_MM:
                                mix = [cls[kt, qb] == 1 for kt in pair]
                                pats = [pat_of.get((kt, qb)) for kt in pair]
                                if (nhalf == 2 and all(mix)
                                        and pats[1] == pats[0] + 1):
                                    nc.vector.tensor_mul(
                                        dst[:], dst[:],
                                        bias_sb[:, pats[0]:pats[0] + 2])
                                else:
                                    for half in range(nhalf):
                                        if mix[half]:
                                            nc.vector.tensor_mul(
                                                dst[:, half], dst[:, half],
                                                bias_sb[:, pats[half]])
                            if ip > 0:
                                nc.vector.tensor_add(S[:, 0:nhalf],
                                                     S[:, 0:nhalf],
                                                     dst[:, 0:nhalf])
                            for half, kt in enumerate(pair):
                                nc.tensor.matmul(
                                    y_ps[:], v_res[:, kt, hsl], dst[:, half],
                                    start=(ip == 0 and half == 0),
                                    stop=(ip == nmm - 1 and half == nhalf - 1))
                        Sf = stage.tile([128, QBS], BF16, tag="sfold", bufs=2)
                        nc.vector.tensor_add(Sf[:], S[:, 0], S[:, 1])
                        # rowsum and its broadcast share one PSUM bank: the
                        # rs value lives in row 0 and is consumed by the
                        # reciprocal before the rb matmul (start=True)
                        # overwrites the bank - the WAR dep enforces order
                        rsrb = psp.tile([128, QBS], F32, tag="rsrb", bufs=1)
                        nc.tensor.matmul(rsrb[0:1, :], ones_sb[:], Sf[:],
                                         start=True, stop=True)
                        rinv = stage.tile([1, QBS], BF16, tag="rinv", bufs=2)
                        nc.vector.reciprocal(rinv[:], rsrb[0:1, :])
                        nc.tensor.matmul(rsrb[:], ones_row[:], rinv[:],
                                         start=True, stop=True)
                        rb_sb = stage.tile([128, QBS], BF16, tag="rbc",
                                           bufs=2)
                        nc.scalar.copy(rb_sb[:], rsrb[:])
                        nc.vector.tensor_mul(y4[:, hb], y_ps[:], rb_sb[:])
                        # per-head spill so the collective only waits on the
                        # last head's small write
                        gi = grp_of[(b, qb)]
                        gqbs = groups[gi][1]
                        off = gqbs.index(qb) * QBS
                        nc.sync.dma_start(
                            y_loc[gi][hb * 128:(hb + 1) * 128,
                                      off:off + QBS],
                            y4[:, hb])
                    if qb == gqbs[-1]:
                        if dist:
                            nc.gpsimd.collective_compute(
                                "AllGather", mybir.AluOpType.bypass,
                                ins=[y_loc[gi].opt()],
                                outs=[y_full[gi].opt()],
                                replica_groups=[list(range(NCORES))],
                            )
                        else:
                            nc.scalar.dma_start(y_full[gi][0:DPC, :],
                                                y_loc[gi][:])

                # ---- phase 3: output projection slice [DPC, T], consuming
                # the gathered chunks in arrival order. Emitted as a
                # generator of small units (one DMA batch or one pair-group)
                # so the final attention chunk can interleave them
                wo_t = []

                def phase3_units():
                    for bb in range(B):
                        for qb in range(QB):
                            gi = grp_of[(bb, qb)]
                            yf = y_full[gi][:].rearrange(
                                "(fo p) t -> p fo t", p=128)
                            tof = groups[gi][1].index(qb) * TN1
                            yc = []
                            for g in range(NG):
                                yg = xsp.tile([128, GF, TN1], BF16,
                                              tag="xchunk",
                                              name=f"yg{bb}_{qb}_{g}")
                                nc.sync.dma_start(
                                    yg[:], yf[:, g * GF:(g + 1) * GF,
                                              tof:tof + TN1])
                                yc.append(yg)
                            for mbp in range(DPC // 256):
                                po = psp.tile([128, 2, TN1], F32, tag="mmp",
                                              bufs=2)
                                for half in range(2):
                                    mb = 2 * mbp + half
                                    for fo in range(FO):
                                        nc.tensor.matmul(
                                            po[:, half], wo_t[mb][:, fo],
                                            yc[fo // GF][:, fo % GF],
                                            start=(fo == 0),
                                            stop=(fo == FO - 1))
                                ot = stage.tile([128, 2, TN1], BF16,
                                                tag="oout", bufs=1)
                                nc.scalar.copy(ot[:], po[:])
                                nc.sync.dma_start(
                                    out_d.rearrange("(m p) t -> p m t", p=128)
                                    [:, 2 * mbp:2 * mbp + 2,
                                     bb * L + qb * TN1:bb * L + (qb + 1) * TN1],
                                    ot[:])
                                yield

                # fine-grained pipeline: each 512-token projection block is
                # immediately followed by its attention chunk and that
                # chunk's all-gather, so the collectives spread across the
                # whole kernel instead of bunching at the end. The last
                # attention chunk interleaves early phase-3 pair-groups
                # (their gathers completed long ago) between its heads.
                p3gen = None
                for b in range(B):
                    for j in range(NB1 // B):
                        phase1_block(b * (NB1 // B) + j)
                        if b == B - 1 and j == NB1 // B - 1:
                            # wo tiles reuse the phase-1 qk weight buffers
                            # (WAR releases once the last projection block's
                            # matmuls finish); issued on the sync queue (idle
                            # here) so their WAR wait + issue time don't
                            # head-of-line block the last chunk's exps on the
                            # Act queue; emitted before the last attention
                            # chunk so the loads overlap it
                            for mb in range(HPC):
                                t3 = wres.tile([128, FO, 128], BF16,
                                               name=f"wmb{mb}")
                                nc.sync.dma_start(
                                    t3[:], wo4_d[mb].rearrange(
                                        "p (fo j) -> p fo j", j=128))
                                wo_t.append(t3)
                            p3gen = phase3_units()
                        attn_chunk(b, j, filler=p3gen)
                for _ in (p3gen or ()):
                    pass

    nc.compile()
    return nc


def _prep_inputs(x, rope, mask, wq, wk, wv, wo):
    x = np.asarray(x, np.float32)
    rope = np.asarray(rope, np.float32)
    mask_b = np.asarray(mask, bool)[0, 0]
    wq = np.asarray(wq, np.float32)
    wk = np.asarray(wk, np.float32)
    wv = np.asarray(wv, np.float32)
    wo = np.asarray(wo, np.float32)

    # rope half-split permutation of q/k output dims
    i = np.arange(HD // 2)
    perm = np.zeros(C, np.int64)
    for h in range(NH):
        perm[h * HD + i] = h * HD + 2 * i
        perm[h * HD + HD // 2 + i] = h * HD + 2 * i + 1
    wq_p, wk_p = wq[perm], wk[perm]

    xT = np.ascontiguousarray(x.reshape(T, C).T).astype(BF16NP)
    cos = rope[:, :, 0].T                      # [64, L]
    sin = rope[:, :, 1].T
    cos1 = np.concatenate([cos, cos], 1)       # [64, T]
    sin1 = np.concatenate([sin, sin], 1)
    cos2 = np.vstack([cos1, cos1])             # [128, T]
    sin2 = np.vstack([-sin1, sin1])
    cs2 = np.ascontiguousarray(
        np.stack([cos2, sin2], axis=1)).astype(BF16NP)   # [128, 2, T]

    maskT_bool = np.ascontiguousarray(mask_b.T)
    _, _, bias_np = _classify(maskT_bool)
    npat = bias_np.shape[1]
    biasm = np.ascontiguousarray(bias_np.reshape(128, npat * QBS))
    ident = np.eye(128, dtype=BF16NP)
    ones = np.ones((128, 1), BF16NP)

    in_maps = []
    FO_, DPC_ = FO, DPC
    for c in range(NCORES):
        sl = slice(c * DPC_, (c + 1) * DPC_)
        A = np.concatenate([wq_p[sl], wk_p[sl]], 0).T          # [C, 1024]
        wqk = np.ascontiguousarray(
            A.reshape(FO_, 128, 8, 128).transpose(2, 1, 0, 3)
            .reshape(8, 128, FO_ * 128)).astype(BF16NP)
        Bv = wv[sl].T                                           # [C, 512]
        wv2 = np.ascontiguousarray(
            Bv.reshape(FO_, 128, DPC_).transpose(1, 0, 2)
            .reshape(128, FO_ * DPC_)).astype(BF16NP)
        Aw = wo[sl].T                                           # [C, 512]
        wo4 = np.ascontiguousarray(
            Aw.reshape(FO_, 128, HPC, 128).transpose(2, 1, 0, 3)
            .reshape(HPC, 128, FO_ * 128)).astype(BF16NP)
        in_maps.append({
            "xt": xT, "wqk": wqk, "wv": wv2, "wo4": wo4,
            "cs2": cs2, "biasm": biasm, "ident": ident, "ones": ones,
        })
    return in_maps, mask_b


def _run_timed(nc, in_maps, k1=8, k2=72, trials=4):
    """Mirror bass2jax.run_bass_via_pjrt multi-core path, but keep inputs
    device-resident and time pipelined executions. Executions are enqueued
    asynchronously (each is a full HW execution; per-core NEFF executions
    serialize on-device), and the per-execution HW time is estimated as the
    marginal cost (T_k2 - T_k1) / (k2 - k1), which cancels the axon RPC
    round-trip latency (~70-100 ms) that would otherwise swamp the ~ms-scale
    kernel. Returns (results, best_ns)."""
    import time
    import jax
    import jax.numpy as jnp
    from jax.experimental.shard_map import shard_map
    from jax.sharding import Mesh, PartitionSpec, NamedSharding
    import concourse.mybir as mybir_
    from concourse import bass2jax as b2j

    b2j.install_neuronx_cc_hook()
    n_cores = len(in_maps)
    partition_name = (nc.partition_id_tensor.name
                      if nc.partition_id_tensor else None)
    in_names, out_names, out_avals, zero_outs = [], [], [], []
    for alloc in nc.m.functions[0].allocations:
        if not isinstance(alloc, mybir_.MemoryLocationSet):
            continue
        name = alloc.memorylocations[0].name
        if alloc.kind == "ExternalInput":
            if name != partition_name:
                in_names.append(name)
        elif alloc.kind == "ExternalOutput":
            shape = tuple(alloc.tensor_shape)
            dtype = mybir_.dt.np(alloc.dtype)
            out_names.append(name)
            out_avals.append(jax.core.ShapedArray(shape, dtype))
            zero_outs.append(np.zeros(shape, dtype))
    n_params = len(in_names)
    all_in = list(in_names) + list(out_names)
    if partition_name is not None:
        all_in.append(partition_name)

    def _body(*args):
        operands = list(args)
        if partition_name is not None:
            operands.append(b2j.partition_id_tensor())
        outs = b2j._bass_exec_p.bind(
            *operands,
            out_avals=tuple(out_avals),
            in_names=tuple(all_in),
            out_names=tuple(out_names),
            lowering_input_output_aliases=(),
            sim_require_finite=True,
            sim_require_nnan=True,
            nc=nc,
        )
        return tuple(outs)

    devices = jax.devices()[:n_cores]
    mesh = Mesh(np.asarray(devices), ("core",))
    in_specs = (PartitionSpec("core"),) * (n_params + len(out_names))
    out_specs = (PartitionSpec("core"),) * len(out_names)
    sharded = jax.jit(shard_map(_body, mesh=mesh, in_specs=in_specs,
                                out_specs=out_specs, check_rep=False),
                      keep_unused=True)
    sh = NamedSharding(mesh, PartitionSpec("core"))
    dev_in = [jax.device_put(
        np.concatenate([np.asarray(in_maps[c][in_names[i]])
                        for c in range(n_cores)], 0), sh)
        for i in range(n_params)]
    dev_zero = [jax.device_put(
        np.zeros((n_cores * z.shape[0], *z.shape[1:]), z.dtype), sh)
        for z in zero_outs]

    out_arrs = sharded(*dev_in, *dev_zero)
    jax.block_until_ready(out_arrs)

    def run_batch(k):
        t0 = time.perf_counter()
        rs = [sharded(*dev_in, *dev_zero) for _ in range(k)]
        jax.block_until_ready(rs)
        return time.perf_counter() - t0

    best = None
    for _ in range(trials):
        ta = run_batch(k1)
        tb = run_batch(k2)
        per_exec = (tb - ta) / (k2 - k1)
        best = per_exec if best is None else min(best, per_exec)
    results = [
        {name: np.asarray(out_arrs[i]).reshape(n_cores, *out_avals[i].shape)[c]
         for i, name in enumerate(out_names)}
        for c in range(n_cores)
    ]
    return results, int(best * 1e9)


def kernel(x, rope, mask, max_seq_length, wq, wk, wv, wo, _trace=False,
           _want_results=False):
    in_maps, mask_b = _prep_inputs(x, rope, mask, wq, wk, wv, wo)
    maskT_bool = np.ascontiguousarray(mask_b.T)

    key = maskT_bool.tobytes()[:4096] + bytes([int(maskT_bool[-1, -1])])
    nc = _CACHED.get(key)
    if nc is None:
        nc = _build(maskT_bool)
        _CACHED[key] = nc

    if _trace:
        results, best_ns = _run_timed(nc, in_maps)
    else:
        res = run_bass_kernel_spmd(nc, in_maps, core_ids=list(range(NCORES)))
        results, best_ns = res.results, None
    outT = np.concatenate([np.asarray(results[c]["out"])
                           for c in range(NCORES)], 0)   # [C, T]
    out = outT.T.astype(np.float32).reshape(B, L, C)
    if _want_results:
        return out, best_ns
    return out


if __name__ == "__main__":
    rng = np.random.default_rng(0)
    x = rng.standard_normal((B, L, C), np.float32)
    print("smoke test build only")


# revision 36
# speedup vs baseline: 1.0854x; 1.0854x over previous
"""Trainium2 Bass kernel: dense transformer attention layer, TP over heads on 8 cores.

Strategy:
  - Shard the 32 heads across 8 cores (4 heads / core). wq/wk/wv column-sharded,
    wo row-sharded; x replicated (transposed + bf16-cast on host).
  - RoPE handled by permuting wq/wk rows on the host into a half-split layout so
    the on-device rotation touches contiguous partition blocks.
  - Fine-grained pipeline: each 512-token projection block is immediately
    followed by its attention chunk (all 4 heads). K is written by RoPE straight
    into resident SBUF tiles, V is copied from PSUM into a resident tile, and Q
    stays in SBUF too (q4 staging tiles double-buffered) - no DRAM round-trips.
  - Attention in transposed layout ST = K^T-major. Score tiles are computed in
    PAIRS into a 2-bank PSUM tile, so one ScalarE exp instruction covers 1024
    columns (halves Act instruction overhead - Act is the attention-phase
    bottleneck). The causal mask is a 0/1 DVE multiply on the exp'd pair
    (USE_BIAS_MM=True instead folds it pre-exp via an I.T @ bias matmul).
    Exp pairs accumulate on the DVE into S; a fold + ones-vector matmul forms
    row sums; the normalizer broadcast is a PE matmul (ones_row x rinv) whose
    PSUM bank is shared with the row-sum (WAR-ordered), freeing a bank for a
    third AV-accumulator buffer.
  - y (bf16) is exchanged in half-batch AllGather chunks, except the final
    chunk which is split per 512-token q block so the last collective is
    0.5 MB/rank and the post-collective tail is one 27us projection slice.
  - Output projection consumes gathered chunks in arrival order, emitted as
    small units; the last attention chunk pulls one unit per head so its
    Act-bound stretches are filled with independent PE work. Output is bf16,
    cast to fp32 on the host.
Timing: `_run_timed` pipelines k executions asynchronously (per-core NEFF
executions serialize on-device) and reports the marginal per-execution time
(T_k2 - T_k1)/(k2 - k1), which cancels the ~70-100 ms axon RPC round-trip.
"""

import sys
import math
import numpy as np

for _p in ("/opt/trn_rl_repo",):
    if _p not in sys.path:
        sys.path.insert(0, _p)

import ml_dtypes  # noqa: E402

import concourse.bass as bass  # noqa: E402
import concourse.mybir as mybir  # noqa: E402
import concourse.tile as tile  # noqa: E402
from concourse import bacc  # noqa: E402
from concourse.bass_utils import run_bass_kernel_spmd  # noqa: E402

BF16 = mybir.dt.bfloat16
F32 = mybir.dt.float32
BF16NP = ml_dtypes.bfloat16

B, L, NH, HD = 2, 2048, 32, 128
C = NH * HD              # 4096
T = B * L                # 4096 tokens total
NCORES = 8
DPC = C // NCORES        # 512 dims per core
HPC = DPC // HD          # 4 heads per core
FO = C // 128            # 32 feature blocks (contraction)
TN1 = 512                # token block for projection phases
NB1 = T // TN1           # 8
QBS = 512                # q block for attention
QB = L // QBS            # 4 per batch
KTILES = L // 128        # 16 k tiles per batch
SCALE = 1.0 / math.sqrt(HD)
MASK_BIAS = -1024.0      # pre-scale additive bias; exp(SCALE*(s-1024)) == 0
# True: apply causal mask as a PE bias-accumulate matmul (I.T @ bias) before
# exp. False: multiply the exp'd pair by a 0/1 mask tile on the DVE.
USE_BIAS_MM = False

_CACHED = {}


def _classify(maskT_bool):
    """cls[kt, qb]: 0 skip, 1 mixed (needs bias), 2 full."""
    cls = np.zeros((KTILES, QB), np.int8)
    pats = {}          # pattern bytes -> index
    pat_of = {}        # (kt, qb) -> pattern index
    for kt in range(KTILES):
        for qb in range(QB):
            m = maskT_bool[kt * 128:(kt + 1) * 128, qb * QBS:(qb + 1) * QBS]
            cls[kt, qb] = 0 if not m.any() else (2 if m.all() else 1)
            if cls[kt, qb] == 1:
                key = m.tobytes()
                if key not in pats:
                    pats[key] = len(pats)
                pat_of[(kt, qb)] = pats[key]
    npat = max(1, len(pats))
    bias = np.zeros((128, npat, QBS), np.float32)
    for key, idx in pats.items():
        m = np.frombuffer(key, bool).reshape(128, QBS)
        bias[:, idx, :] = (np.where(m, 0.0, MASK_BIAS) if USE_BIAS_MM
                           else m.astype(np.float32))
    return cls, pat_of, bias.astype(BF16NP)


def _build(maskT_bool, dist=True):
    """maskT_bool: [L, L] bool, maskT[k, q] = attend(q -> k)."""
    cls, pat_of, bias_np = _classify(maskT_bool)
    npat = bias_np.shape[1]

    nc = bacc.Bacc("TRN2", target_bir_lowering=False, debug=False,
                   num_devices=NCORES)

    xt = nc.dram_tensor("xt", [C, T], BF16, kind="ExternalInput")
    wqk_d = nc.dram_tensor("wqk", [2 * HPC, 128, FO * 128], BF16,
                           kind="ExternalInput")
    wv_d = nc.dram_tensor("wv", [128, FO * DPC], BF16, kind="ExternalInput")
    wo4_d = nc.dram_tensor("wo4", [HPC, 128, FO * 128], BF16,
                           kind="ExternalInput")
    cs_d = nc.dram_tensor("cs2", [128, 2, T], BF16, kind="ExternalInput")
    bias_d = nc.dram_tensor("biasm", [128, npat * QBS], BF16,
                            kind="ExternalInput")
    ident_d = nc.dram_tensor("ident", [128, 128], BF16, kind="ExternalInput")
    ones_d = nc.dram_tensor("ones", [128, 1], BF16, kind="ExternalInput")
    out_d = nc.dram_tensor("out", [DPC, T], BF16, kind="ExternalOutput")

    Exp = mybir.ActivationFunctionType.Exp

    with tile.TileContext(nc) as tc, nc.allow_low_precision(
            reason="bf16 rope temps / softmax-normalizer broadcast / bf16 "
                   "output; rel-err budget is 2e-2 and matmul accumulation "
                   "stays fp32"):
        with (
            tc.tile_pool(name="stage", bufs=2) as stage,
            tc.tile_pool(name="psum", bufs=1, space="PSUM") as psp,
            tc.tile_pool(name="dram", bufs=1, space="DRAM") as dram,
        ):
            # y is exchanged in half-batch chunks (2 q blocks = 1024 tokens),
            # except the LAST chunk which is split per-q-block so the final
            # collective is small and the post-collective tail is a single
            # 27us projection slice
            groups = []           # (b, [qb...])
            for b in range(B):
                for h in range(QB // 2):
                    qbs = [2 * h, 2 * h + 1]
                    if b == B - 1 and h == QB // 2 - 1:
                        groups.append((b, [qbs[0]]))
                        groups.append((b, [qbs[1]]))
                    else:
                        groups.append((b, qbs))
            grp_of = {(b, qb): gi for gi, (b, qbs) in enumerate(groups)
                      for qb in qbs}
            y_loc = [dram.tile([DPC, len(qbs) * QBS], BF16,
                               name=f"y_loc{gi}")
                     for gi, (b, qbs) in enumerate(groups)]
            y_full = [dram.tile([C, len(qbs) * QBS], BF16,
                                addr_space="Shared", name=f"y_full{gi}")
                      for gi, (b, qbs) in enumerate(groups)]
            xt_r = xt.rearrange("(fo p) t -> p fo t", p=128)

            with (
                tc.tile_pool(name="wres", bufs=1) as wres,
                tc.tile_pool(name="xs", bufs=5) as xsp,
                tc.tile_pool(name="qy", bufs=2) as qyp,
                tc.tile_pool(name="ptp", bufs=3) as ptp,
            ):
                # ---- phase 1: QKV projection + RoPE
                # emission order = DMA FIFO order: interleave the first token
                # block's x chunks with the weight tiles so the first matmul
                # group starts after ~2MB of DMA, not 16MB
                GF = 8            # fo per x chunk
                NG = FO // GF     # 4 chunks per token block

                def load_x(n):
                    tsl = slice(n * TN1, (n + 1) * TN1)
                    xc = []
                    for g in range(NG):
                        xg = xsp.tile([128, GF, TN1], BF16, tag="xchunk",
                                      name=f"xg{n}_{g}")
                        nc.sync.dma_start(
                            xg[:], xt_r[:, g * GF:(g + 1) * GF, tsl])
                        xc.append(xg)
                    cs_sb = stage.tile([128, 2, TN1], BF16, tag="csl", bufs=2,
                                       name=f"cs{n}")
                    nc.sync.dma_start(cs_sb[:], cs_d[:, :, tsl])
                    return xc, cs_sb

                # prologue DMA order: the first pair-group's matmul chain
                # needs x chunks 0..3 + wmb0 (+ cs0 for rope, wmb1 for the
                # second half) - load exactly that first, everything else after
                w_mb = []
                x0c = []
                x0g0 = xsp.tile([128, GF, TN1], BF16, tag="xchunk", name="xg0_0")
                nc.sync.dma_start(x0g0[:], xt_r[:, 0:GF, 0:TN1])
                x0c.append(x0g0)
                t0 = wres.tile([128, FO, 128], BF16, name="wmb0")
                nc.sync.dma_start(t0[:], wqk_d[0].rearrange(
                    "p (fo j) -> p fo j", j=128))
                w_mb.append(t0)
                for g in range(1, NG):
                    xg = xsp.tile([128, GF, TN1], BF16, tag="xchunk",
                                  name=f"xg0_{g}")
                    nc.sync.dma_start(
                        xg[:], xt_r[:, g * GF:(g + 1) * GF, 0:TN1])
                    x0c.append(xg)
                cs0 = stage.tile([128, 2, TN1], BF16, tag="csl",
                                 bufs=2, name="cs0")
                nc.sync.dma_start(cs0[:], cs_d[:, :, 0:TN1])
                for mb in range(1, 2 * HPC):
                    t = wres.tile([128, FO, 128], BF16, name=f"wmb{mb}")
                    nc.sync.dma_start(t[:], wqk_d[mb].rearrange(
                        "p (fo j) -> p fo j", j=128))
                    w_mb.append(t)
                w_v = wres.tile([128, FO, DPC], BF16)
                nc.sync.dma_start(w_v[:], wv_d.rearrange("p (fo j) -> p fo j", j=DPC))
                ones_sb = wres.tile([128, 1], BF16)
                nc.sync.dma_start(ones_sb[:], ones_d[:, :])
                bias_sb = wres.tile([128, npat, QBS], BF16)
                nc.sync.dma_start(bias_sb[:], bias_d.rearrange(
                    "p (np q) -> p np q", q=QBS))
                ident_sb = wres.tile([128, 128], BF16)
                nc.sync.dma_start(ident_sb[:], ident_d[:, :])
                # [1,128] all-ones lhsT used to broadcast the softmax
                # normalizer across partitions on the PE
                ones_row = wres.tile([1, 128], BF16)
                nc.vector.memset(ones_row[:], 1.0)
                # K and V stay SBUF-resident for the current batch
                k_res = [wres.tile([128, L], BF16, name=f"kres{hb}")
                         for hb in range(HPC)]
                v_res = wres.tile([128, KTILES, DPC], BF16)

                q4 = {}    # block n -> q staging tile [128, HPC, TN1]

                def phase1_block(n):
                    b, j = divmod(n, NB1 // B)
                    wtsl = slice(j * TN1, (j + 1) * TN1)
                    if n == 0:
                        xc, cs_sb = x0c, cs0
                    else:
                        xc, cs_sb = load_x(n)
                    cos_sb = cs_sb[:, 0]
                    sin_sb = cs_sb[:, 1]
                    q4[n] = qyp.tile([128, HPC, TN1], BF16, tag="q4",
                                     name=f"q4_{n}")
                    # q/k head-blocks in PSUM pairs (2 banks per tile)
                    for mbp in range(HPC):
                        ps = psp.tile([128, 2, TN1], F32, tag="mmp", bufs=2)
                        for half in range(2):
                            mb = 2 * mbp + half
                            for fo in range(FO):
                                nc.tensor.matmul(ps[:, half], w_mb[mb][:, fo],
                                                 xc[fo // GF][:, fo % GF],
                                                 start=(fo == 0),
                                                 stop=(fo == FO - 1))
                        for half in range(2):
                            mb = 2 * mbp + half
                            # rope: out = p*cos2 + rot(p)*sin2 (top half of
                            # sin2 negated on host)
                            tmp = stage.tile([128, TN1], BF16, tag="ropetmp", bufs=1)
                            rot = stage.tile([128, TN1], BF16, tag="roperot", bufs=1)
                            nc.vector.tensor_mul(tmp[:], ps[:, half], cos_sb)
                            nc.vector.tensor_mul(rot[0:64], ps[64:128, half],
                                                 sin_sb[0:64])
                            nc.vector.tensor_mul(rot[64:128], ps[0:64, half],
                                                 sin_sb[64:128])
                            if mb < HPC:   # Q head-block: into SBUF staging
                                nc.vector.tensor_add(q4[n][:, mb], tmp[:],
                                                     rot[:])
                            else:          # K head-block: into k_res
                                nc.vector.tensor_add(
                                    k_res[mb - HPC][:, wtsl], tmp[:], rot[:])
                    for tb in range(TN1 // 128):
                        psv = psp.tile([128, DPC], F32, tag="acc", bufs=3)
                        for fo in range(FO):
                            nc.tensor.matmul(
                                psv[:],
                                xc[fo // GF][:, fo % GF, tb * 128:(tb + 1) * 128],
                                w_v[:, fo], start=(fo == 0), stop=(fo == FO - 1))
                        nc.any.tensor_copy(v_res[:, j * 4 + tb, :], psv[:])

                # ---- attention chunk (b, qb): all 4 heads for one 512-token
                # q block; score tiles in pairs, one exp per pair, causal mask
                # as a 0/1 DVE multiply (or bias-accumulate matmul). `filler`
                # is an optional generator pulled once per head to interleave
                # independent PE work (phase-3 units) into the emission
                def attn_chunk(b, qb, filler=None):
                    n = b * (NB1 // B) + qb
                    acts = [kt for kt in range(KTILES) if cls[kt, qb] > 0]
                    npair_ = len(acts) // 2
                    rem = len(acts) - 2 * npair_
                    y4 = qyp.tile([128, HPC, QBS], BF16, tag="y4", bufs=1,
                                  name=f"y4_{b}_{qb}")
                    for hb in range(HPC):
                        if filler is not None:
                            next(filler, None)
                        hsl = slice(hb * 128, (hb + 1) * 128)
                        y_ps = psp.tile([128, QBS], F32, tag="acc", bufs=3)
                        S = ptp.tile([128, 2, QBS], BF16, tag="ssum", bufs=2)
                        nmm = npair_ + rem
                        if rem and nmm == 1:
                            nc.vector.memset(S[:, 1], 0.0)
                        for ip in range(nmm):
                            pair = [acts[2 * ip]] if (rem and ip == nmm - 1) \
                                else acts[2 * ip:2 * ip + 2]
                            st = psp.tile([128, 2, QBS], F32, tag="mmp",
                                          bufs=2)
                            for half, kt in enumerate(pair):
                                mixed = USE_BIAS_MM and cls[kt, qb] == 1
                                nc.tensor.matmul(
                                    st[:, half],
                                    k_res[hb][:, kt * 128:(kt + 1) * 128],
                                    q4[n][:, hb], start=True, stop=not mixed)
                                if mixed:
                                    nc.tensor.matmul(
                                        st[:, half], ident_sb[:],
                                        bias_sb[:, pat_of[(kt, qb)]],
                                        start=False, stop=True)
                            nhalf = len(pair)
                            dst = S if ip == 0 else ptp.tile(
                                [128, 2, QBS], BF16, tag="pt", bufs=2)
                            nc.scalar.activation(dst[:, 0:nhalf],
                                                 st[:, 0:nhalf], Exp,
                                                 scale=SCALE)
                            if not USE_BIAS_MM:
                                mix = [cls[kt, qb] == 1 for kt in pair]
                                pats = [pat_of.get((kt, qb)) for kt in pair]
                                if (nhalf == 2 and all(mix)
                                        and pats[1] == pats[0] + 1):
                                    nc.vector.tensor_mul(
                                        dst[:], dst[:],
                                        bias_sb[:, pats[0]:pats[0] + 2])
                                else:
                                    for half in range(nhalf):
                                        if mix[half]:
                                            nc.vector.tensor_mul(
                                                dst[:, half], dst[:, half],
                                                bias_sb[:, pats[half]])
                            if ip > 0:
                                nc.vector.tensor_add(S[:, 0:nhalf],
                                                     S[:, 0:nhalf],
                                                     dst[:, 0:nhalf])
                            for half, kt in enumerate(pair):
                                nc.tensor.matmul(
                                    y_ps[:], v_res[:, kt, hsl], dst[:, half],
                                    start=(ip == 0 and half == 0),
                                    stop=(ip == nmm - 1 and half == nhalf - 1))
                        Sf = stage.tile([128, QBS], BF16, tag="sfold", bufs=2)
                        nc.vector.tensor_add(Sf[:], S[:, 0], S[:, 1])
                        # rowsum and its broadcast share one PSUM bank: the
                        # rs value lives in row 0 and is consumed by the
                        # reciprocal before the rb matmul (start=True)
                        # overwrites the bank - the WAR dep enforces order
                        rsrb = psp.tile([128, QBS], F32, tag="rsrb", bufs=1)
                        nc.tensor.matmul(rsrb[0:1, :], ones_sb[:], Sf[:],
                                         start=True, stop=True)
                        rinv = stage.tile([1, QBS], BF16, tag="rinv", bufs=2)
                        nc.vector.reciprocal(rinv[:], rsrb[0:1, :])
                        nc.tensor.matmul(rsrb[:], ones_row[:], rinv[:],
                                         start=True, stop=True)
                        rb_sb = stage.tile([128, QBS], BF16, tag="rbc",
                                           bufs=2)
                        nc.scalar.copy(rb_sb[:], rsrb[:])
                        nc.vector.tensor_mul(y4[:, hb], y_ps[:], rb_sb[:])
                        # per-head spill so the collective only waits on the
                        # last head's small write
                        gi = grp_of[(b, qb)]
                        gqbs = groups[gi][1]
                        off = gqbs.index(qb) * QBS
                        nc.sync.dma_start(
                            y_loc[gi][hb * 128:(hb + 1) * 128,
                                      off:off + QBS],
                            y4[:, hb])
                    if qb == gqbs[-1]:
                        if dist:
                            nc.gpsimd.collective_compute(
                                "AllGather", mybir.AluOpType.bypass,
                                ins=[y_loc[gi].opt()],
                                outs=[y_full[gi].opt()],
                                replica_groups=[list(range(NCORES))],
                            )
                        else:
                            nc.scalar.dma_start(y_full[gi][0:DPC, :],
                                                y_loc[gi][:])

                # ---- phase 3: output projection slice [DPC, T], consuming
                # the gathered chunks in arrival order. Emitted as a
                # generator of small units (one DMA batch or one pair-group)
                # so the final attention chunk can interleave them
                wo_t = []

                def phase3_units():
                    for bb in range(B):
                        for qb in range(QB):
                            gi = grp_of[(bb, qb)]
                            yf = y_full[gi][:].rearrange(
                                "(fo p) t -> p fo t", p=128)
                            tof = groups[gi][1].index(qb) * TN1
                            yc = []
                            for g in range(NG):
                                yg = xsp.tile([128, GF, TN1], BF16,
                                              tag="xchunk",
                                              name=f"yg{bb}_{qb}_{g}")
                                nc.sync.dma_start(
                                    yg[:], yf[:, g * GF:(g + 1) * GF,
                                              tof:tof + TN1])
                                yc.append(yg)
                            for mbp in range(DPC // 256):
                                po = psp.tile([128, 2, TN1], F32, tag="mmp",
                                              bufs=2)
                                for half in range(2):
                                    mb = 2 * mbp + half
                                    for fo in range(FO):
                                        nc.tensor.matmul(
                                            po[:, half], wo_t[mb][:, fo],
                                            yc[fo // GF][:, fo % GF],
                                            start=(fo == 0),
                                            stop=(fo == FO - 1))
                                ot = stage.tile([128, 2, TN1], BF16,
                                                tag="oout", bufs=1)
                                nc.scalar.copy(ot[:], po[:])
                                nc.sync.dma_start(
                                    out_d.rearrange("(m p) t -> p m t", p=128)
                                    [:, 2 * mbp:2 * mbp + 2,
                                     bb * L + qb * TN1:bb * L + (qb + 1) * TN1],
                                    ot[:])
                                yield

                # fine-grained pipeline: each 512-token projection block is
                # immediately followed by its attention chunk and that
                # chunk's all-gather, so the collectives spread across the
                # whole kernel instead of bunching at the end. The last
                # attention chunk interleaves early phase-3 pair-groups
                # (their gathers completed long ago) between its heads.
                p3gen = None
                for b in range(B):
                    for j in range(NB1 // B):
                        phase1_block(b * (NB1 // B) + j)
                        if b == B - 1 and j == NB1 // B - 1:
                            # wo tiles reuse the phase-1 qk weight buffers
                            # (WAR releases once the last projection block's
                            # matmuls finish); issued on the sync queue (idle
                            # here) so their WAR wait + issue time don't
                            # head-of-line block the last chunk's exps on the
                            # Act queue; emitted before the last attention
                            # chunk so the loads overlap it
                            for mb in range(HPC):
                                t3 = wres.tile([128, FO, 128], BF16,
                                               name=f"wmb{mb}")
                                nc.sync.dma_start(
                                    t3[:], wo4_d[mb].rearrange(
                                        "p (fo j) -> p fo j", j=128))
                                wo_t.append(t3)
                            p3gen = phase3_units()
                        attn_chunk(b, j, filler=p3gen)
                for _ in (p3gen or ()):
                    pass

    nc.compile()
    return nc


def _prep_inputs(x, rope, mask, wq, wk, wv, wo):
    x = np.asarray(x, np.float32)
    rope = np.asarray(rope, np.float32)
    mask_b = np.asarray(mask, bool)[0, 0]
    wq = np.asarray(wq, np.float32)
    wk = np.asarray(wk, np.float32)
    wv = np.asarray(wv, np.float32)
    wo = np.asarray(wo, np.float32)

    # rope half-split permutation of q/k output dims
    i = np.arange(HD // 2)
    perm = np.zeros(C, np.int64)
    for h in range(NH):
        perm[h * HD + i] = h * HD + 2 * i
        perm[h * HD + HD // 2 + i] = h * HD + 2 * i + 1
    wq_p, wk_p = wq[perm], wk[perm]

    xT = np.ascontiguousarray(x.reshape(T, C).T).astype(BF16NP)
    cos = rope[:, :, 0].T                      # [64, L]
    sin = rope[:, :, 1].T
    cos1 = np.concatenate([cos, cos], 1)       # [64, T]
    sin1 = np.concatenate([sin, sin], 1)
    cos2 = np.vstack([cos1, cos1])             # [128, T]
    sin2 = np.vstack([-sin1, sin1])
    cs2 = np.ascontiguousarray(
        np.stack([cos2, sin2], axis=1)).astype(BF16NP)   # [128, 2, T]

    maskT_bool = np.ascontiguousarray(mask_b.T)
    _, _, bias_np = _classify(maskT_bool)
    npat = bias_np.shape[1]
    biasm = np.ascontiguousarray(bias_np.reshape(128, npat * QBS))
    ident = np.eye(128, dtype=BF16NP)
    ones = np.ones((128, 1), BF16NP)

    in_maps = []
    FO_, DPC_ = FO, DPC
    for c in range(NCORES):
        sl = slice(c * DPC_, (c + 1) * DPC_)
        A = np.concatenate([wq_p[sl], wk_p[sl]], 0).T          # [C, 1024]
        wqk = np.ascontiguousarray(
            A.reshape(FO_, 128, 8, 128).transpose(2, 1, 0, 3)
            .reshape(8, 128, FO_ * 128)).astype(BF16NP)
        Bv = wv[sl].T                                           # [C, 512]
        wv2 = np.ascontiguousarray(
            Bv.reshape(FO_, 128, DPC_).transpose(1, 0, 2)
            .reshape(128, FO_ * DPC_)).astype(BF16NP)
        Aw = wo[sl].T                                           # [C, 512]
        wo4 = np.ascontiguousarray(
            Aw.reshape(FO_, 128, HPC, 128).transpose(2, 1, 0, 3)
            .reshape(HPC, 128, FO_ * 128)).astype(BF16NP)
        in_maps.append({
            "xt": xT, "wqk": wqk, "wv": wv2, "wo4": wo4,
            "cs2": cs2, "biasm": biasm, "ident": ident, "ones": ones,
        })
    return in_maps, mask_b


def _run_timed(nc, in_maps, k1=8, k2=72, trials=4):
    """Mirror bass2jax.run_bass_via_pjrt multi-core path, but keep inputs
    device-resident and time pipelined executions. Executions are enqueued
    asynchronously (each is a full HW execution; per-core NEFF executions
    serialize on-device), and the per-execution HW time is estimated as the
    marginal cost (T_k2 - T_k1) / (k2 - k1), which cancels the axon RPC
    round-trip latency (~70-100 ms) that would otherwise swamp the ~ms-scale
    kernel. Returns (results, best_ns)."""
    import time
    import jax
    import jax.numpy as jnp
    from jax.experimental.shard_map import shard_map
    from jax.sharding import Mesh, PartitionSpec, NamedSharding
    import concourse.mybir as mybir_
    from concourse import bass2jax as b2j

    b2j.install_neuronx_cc_hook()
    n_cores = len(in_maps)
    partition_name = (nc.partition_id_tensor.name
                      if nc.partition_id_tensor else None)
    in_names, out_names, out_avals, zero_outs = [], [], [], []
    for alloc in nc.m.functions[0].allocations:
        if not isinstance(alloc, mybir_.MemoryLocationSet):
            continue
        name = alloc.memorylocations[0].name
        if alloc.kind == "ExternalInput":
            if name != partition_name:
                in_names.append(name)
        elif alloc.kind == "ExternalOutput":
            shape = tuple(alloc.tensor_shape)
            dtype = mybir_.dt.np(alloc.dtype)
            out_names.append(name)
            out_avals.append(jax.core.ShapedArray(shape, dtype))
            zero_outs.append(np.zeros(shape, dtype))
    n_params = len(in_names)
    all_in = list(in_names) + list(out_names)
    if partition_name is not None:
        all_in.append(partition_name)

    def _body(*args):
        operands = list(args)
        if partition_name is not None:
            operands.append(b2j.partition_id_tensor())
        outs = b2j._bass_exec_p.bind(
            *operands,
            out_avals=tuple(out_avals),
            in_names=tuple(all_in),
            out_names=tuple(out_names),
            lowering_input_output_aliases=(),
            sim_require_finite=True,
            sim_require_nnan=True,
            nc=nc,
        )
        return tuple(outs)

    devices = jax.devices()[:n_cores]
    mesh = Mesh(np.asarray(devices), ("core",))
    in_specs = (PartitionSpec("core"),) * (n_params + len(out_names))
    out_specs = (PartitionSpec("core"),) * len(out_names)
    sharded = jax.jit(shard_map(_body, mesh=mesh, in_specs=in_specs,
                                out_specs=out_specs, check_rep=False),
                      keep_unused=True)
    sh = NamedSharding(mesh, PartitionSpec("core"))
    dev_in = [jax.device_put(
        np.concatenate([np.asarray(in_maps[c][in_names[i]])
                        for c in range(n_cores)], 0), sh)
        for i in range(n_params)]
    dev_zero = [jax.device_put(
        np.zeros((n_cores * z.shape[0], *z.shape[1:]), z.dtype), sh)
        for z in zero_outs]

    out_arrs = sharded(*dev_in, *dev_zero)
    jax.block_until_ready(out_arrs)

    def run_batch(k):
        t0 = time.perf_counter()
        rs = [sharded(*dev_in, *dev_zero) for _ in range(k)]
        jax.block_until_ready(rs)
        return time.perf_counter() - t0

    best = None
    for _ in range(trials):
        ta = run_batch(k1)
        tb = run_batch(k2)
        per_exec = (tb - ta) / (k2 - k1)
        best = per_exec if best is None else min(best, per_exec)
    results = [
        {name: np.asarray(out_arrs[i]).reshape(n_cores, *out_avals[i].shape)[c]
         for i, name in enumerate(out_names)}
        for c in range(n_cores)
    ]
    return results, int(best * 1e9)


def kernel(x, rope, mask, max_seq_length, wq, wk, wv, wo, _trace=False,
           _want_results=False):
    in_maps, mask_b = _prep_inputs(x, rope, mask, wq, wk, wv, wo)
    maskT_bool = np.ascontiguousarray(mask_b.T)

    key = maskT_bool.tobytes()[:4096] + bytes([int(maskT_bool[-1, -1])])
    nc = _CACHED.get(key)
    if nc is None:
        nc = _build(maskT_bool)
        _CACHED[key] = nc

    if _trace:
        results, best_ns = _run_timed(nc, in_maps)
    else:
        res = run_bass_kernel_spmd(nc, in_maps, core_ids=list(range(NCORES)))
        results, best_ns = res.results, None
    outT = np.concatenate([np.asarray(results[c]["out"])
                           for c in range(NCORES)], 0)   # [C, T]
    out = outT.T.astype(np.float32).reshape(B, L, C)
    if _want_results:
        return out, best_ns
    return out


if __name__ == "__main__":
    rng = np.random.default_rng(0)
    x = rng.standard_normal((B, L, C), np.float32)
    print("smoke test build only")
